# revision 2
# baseline (speedup 1.0000x reference)
"""Geometric-product 3D conv (Cl(3,0) GA conv) on 8 Trainium2 NeuronCores.

Problem: x[B=8, Cin=8, I=8, 48,48,48], W[3,3,3, Cin=8, Cout=8, J=8], b[8,8],
G[8,8,8] ->  out[B=8, Cout=8, K=8, 46,46,46]   (VALID 3d conv after folding
the geometric-product table G into the weights).

Strategy (v2, bf16):
  * Fold G into W on host -> dense conv kernel Kfold[o'=64, c'=64, 3,3,3].
  * Data parallel: one batch element per NeuronCore (8 cores).
  * Conv as matmul with d-parity packing: SBUF x layout has partitions
    p = c'*2 + (d mod 2), so each "d-pair" tile [128, 2368] holds two
    adjacent depth slices (2304 payload + 64 tail from the next d-row so
    shifted reads stay in bounds; the tail only lands in cropped garbage
    columns).  Output packs (o', d mod 2) on its 128 partitions.  One
    output d-pair needs 2 accumulation steps x 9 (l,v) kernel offsets
    = 18 matmuls of [K=128, M=128] x [128, N] per PSUM tile.
  * bf16 operands (x, folded weights): same 1 cycle/row PE stream rate as
    fp32r, but enables fast-weight-load + background LDWEIGHTS overlap
    (fp32r self-loads 4-byte weights serially, ~2.5x slower per matmul on
    HW) and halves input DMA.  PSUM/bias/output stay fp32.  Measured
    relative error vs fp32 reference ~1e-3, comfortably under the 2e-2
    gate (fp32r variant was 1.6e-4 at ~1.19 ms/core).
  * DMA: ONE input DMA per d-pair tile (tail pad baked into the host
    layout, 4.6KB/partition contiguous both sides) and ONE output DMA per
    output d-pair (the 5 PSUM h-chunks are drained by the DVE bias-add
    into a single contiguous [128, 46*46] SBUF tile; 8.5KB/partition
    contiguous both sides).  The v1 kernel cropped w=46 of 48 during
    DMA-out, producing 184B descriptor runs (under the 512B line-rate
    minimum) and ~56 GB/s effective bandwidth; the crop now happens in
    the DVE drain instead.  Input DMAs issue on the sync-engine HWDGE
    ring, output DMAs on the scalar-engine ring.
  * (h,w) plane is computed on the full 48-wide grid (stride alignment
    with the input) in PSUM-bank-sized chunks of N=nh*48<=480.
"""

import sys

import numpy as np

sys.path.insert(0, "/opt/trn_rl_repo")

_PROGRAM = None

# h-row chunks of the 46-row output plane; N = nh*48 (<=512 fp32 PSUM bank)
_CHUNKS = [(0, 10), (10, 10), (20, 10), (30, 10), (40, 6)]


def _build_program(chunks=None, xp_bufs=6, oq_bufs=3, repeat=0):
    import contextlib

    import concourse.bacc as bacc
    import concourse.mybir as mybir
    from concourse import tile

    chunks = chunks or _CHUNKS
    f32 = mybir.dt.float32
    bf16 = mybir.dt.bfloat16

    nc = bacc.Bacc(None, target_bir_lowering=False)
    # x: [c'=64, dpair j=0..23, dp=0/1, 2304 payload + 64 tail]
    x_in = nc.declare_dram_parameter("x", [64, 24, 2, 2368], bf16, isOutput=False)
    wt_in = nc.declare_dram_parameter("wt", [128, 18 * 128], bf16, isOutput=False)
    b_in = nc.declare_dram_parameter("bias", [128, 1], f32, isOutput=False)
    # [o', d%2, d//2, h*w]: leading (o', dp) merge to the 128 SBUF partitions
    # in one DMA AP dim; host untangles d = 2q+dp.
    out_ext = nc.declare_dram_parameter("out", [64, 2, 23, 46 * 46], f32, isOutput=True)

    with tile.TileContext(nc) as tc:
        with (
            tc.tile_pool(name="wt", bufs=1) as wtp,
            tc.tile_pool(name="xp", bufs=xp_bufs) as xpp,
            tc.tile_pool(name="ps", bufs=1, space="PSUM") as psp,
            tc.tile_pool(name="oq", bufs=oq_bufs) as oqp,
            tc.tile_pool(name="bias", bufs=1) as bp,
        ):
            # repeat>0 wraps the whole body in a HW loop (benchmarking only)
            rep_ctx = tc.For_i(0, repeat, 1) if repeat else contextlib.nullcontext()
            with rep_ctx:
                _emit_body(nc, tc, chunks, wtp, xpp, psp, oqp, bp,
                           x_in, wt_in, b_in, out_ext, f32, bf16)
    nc.finalize()
    return nc


def _emit_body(nc, tc, chunks, wtp, xpp, psp, oqp, bp,
               x_in, wt_in, b_in, out_ext, f32, bf16):
    wt = wtp.tile([128, 18 * 128], bf16)
    nc.sync.dma_start(out=wt[:], in_=wt_in[:])
    bias = bp.tile([128, 1], f32)
    nc.sync.dma_start(out=bias[:], in_=b_in[:])

    xp = {}

    def load_xpair(j):
        t = xpp.tile([128, 2368], bf16, tag="xp")
        nc.sync.dma_start(out=t[:], in_=x_in[:, j, :, :])
        xp[j] = t

    load_xpair(0)
    load_xpair(1)

    for Q in range(23):
        if Q + 2 <= 23:
            load_xpair(Q + 2)
        oq = oqp.tile([128, 46 * 46], f32, tag="oq")
        for ci, (h0, nh) in enumerate(chunks):
            N = nh * 48
            f0 = h0 * 48
            ps = psp.tile([128, nh, 48], f32, tag=f"ps{ci}")
            k = 0
            for s in (0, 1):
                rhs_t = xp[Q + s]
                for l in range(3):
                    for v in range(3):
                        off = f0 + l * 48 + v
                        nc.tensor.matmul(
                            ps[:],
                            lhsT=wt[:, 128 * k : 128 * (k + 1)],
                            rhs=rhs_t[:, off : off + N],
                            start=(k == 0),
                            stop=(k == 17),
                        )
                        k += 1
            # bias-add drain; crops the 2 garbage w columns (48 -> 46)
            nc.vector.tensor_scalar_add(
                out=oq[:, h0 * 46 : (h0 + nh) * 46],
                in0=ps[:, :, 0:46],
                scalar1=bias[:],
            )
        nc.scalar.dma_start(out=out_ext[:, :, Q, :], in_=oq[:])


def _get_program():
    global _PROGRAM
    if _PROGRAM is None:
        _PROGRAM = _build_program()
    return _PROGRAM


def _prepare_host_inputs(x, W, b, G):
    import ml_dtypes

    bf16 = ml_dtypes.bfloat16
    B = x.shape[0]
    # Fold GA product table into the conv kernel:
    # out[b,o,k,d,h,w] = sum G[i,j,k] x[b,c,i,...] W[m,l,v,c,o,j]
    Wt = np.einsum("ijk,mlvcoj->okcimlv", G, W).astype(np.float32)
    Kfold = np.ascontiguousarray(Wt.reshape(64, 64, 3, 3, 3))  # [o', c', m, l, v]

    # 18 stationary matrices: lhsT[k_in = c'*2+dpi, p_out = o'*2+dpo]
    WBIG = np.zeros((128, 18, 128), np.float32)
    L = np.zeros((64, 2, 64, 2), np.float32)  # [c', dpi, o', dpo]
    for s in (0, 1):
        for l in range(3):
            for v in range(3):
                k = s * 9 + l * 3 + v
                L[:] = 0.0
                for dpi in (0, 1):
                    for dpo in (0, 1):
                        m = 2 * s + dpi - dpo
                        if 0 <= m <= 2:
                            L[:, dpi, :, dpo] = Kfold[:, :, m, l, v].T
                WBIG[:, k, :] = L.reshape(128, 128)
    wt_arr = np.ascontiguousarray(WBIG.reshape(128, 18 * 128)).astype(bf16)

    bias_arr = np.repeat(b.reshape(64).astype(np.float32), 2).reshape(128, 1)
    bias_arr = np.ascontiguousarray(bias_arr)

    # x -> [c'=64, j=0..23, dp, 2368]: row 2j+dp (2304) ++ row 2j+dp+1 [0:64]
    # (d row 48 is zero padding so the last tail stays in bounds)
    zrow = np.zeros((64, 1, 2304), np.float32)
    xs = []
    for i in range(B):
        flat = np.concatenate([x[i].reshape(64, 48, 2304), zrow], axis=1)
        flat = np.ascontiguousarray(flat).reshape(64, 49 * 2304)
        st = flat.strides
        xi = np.lib.stride_tricks.as_strided(
            flat, shape=(64, 24, 2, 2368),
            strides=(st[0], 2 * 2304 * 4, 2304 * 4, 4),
        )
        xs.append(np.ascontiguousarray(xi).astype(bf16))
    return xs, wt_arr, bias_arr


def kernel(**inputs):
    from concourse.bass_utils import run_bass_kernel_spmd

    x = np.asarray(inputs["x"], np.float32)
    W = np.asarray(inputs["W"], np.float32)
    b = np.asarray(inputs["b"], np.float32)
    G = np.asarray(inputs["G"], np.float32)

    xs, wt_arr, bias_arr = _prepare_host_inputs(x, W, b, G)
    nc = _get_program()
    in_maps = [{"x": xs[i], "wt": wt_arr, "bias": bias_arr} for i in range(8)]
    res = run_bass_kernel_spmd(nc, in_maps, list(range(8)))
    out = np.stack([_unpack_out(res.results[i]["out"]) for i in range(8)], axis=0)
    return out.reshape(8, 8, 8, 46, 46, 46)


def _unpack_out(arr):
    # [o', dp, q, h*w] -> [o', d=2q+dp, h, w]
    return np.ascontiguousarray(
        np.asarray(arr, np.float32).reshape(64, 2, 23, 46, 46).transpose(0, 2, 1, 3, 4)
    ).reshape(64, 46, 46, 46)
